# revision 1
# baseline (speedup 1.0000x reference)
"""Trainium2 Bass kernel for gnn_message_passing (nn_BFR_28089086116615).

Sharding: receiver axis i (G=4096 -> 8 cores x 512). Host pre-transposes the
edge matrices and folds the {coef, 1} gate weights in bf16: wT[j, i]. On
device, sigma^T is computed natively in [j-partition, i-free] layout (ACT
sigmoid, per-partition bias = s_src[j-chunk], input = broadcast s_dst row),
gated by wT on DVE (bf16 2x), and contracted on PE with stationary weights
[1 | h] so the receiver rowsum lands in psum row 0. s_src comes from a DVE
multiply+reduce over the natural-layout h (no PE involvement). BatchNorm is
per-gene -> fully local; two per-batch AllGathers of normalized h between the
blocks so block-2 can start on batch 0 while batch 1 is still in flight.
"""
import sys
sys.path.insert(0, "/opt/trn_rl_repo")
import numpy as np
import ml_dtypes

import concourse.bass as bass
import concourse.bacc as bacc
import concourse.mybir as mybir
import concourse.tile as tile
from concourse.bass_utils import run_bass_kernel_spmd

NC = 8
B, G, NI, H, NO = 2, 4096, 8, 32, 32
GL = G // NC              # 512 local receivers per core
LCH = GL // 128           # 4 local chunks
NCH = G // 128            # 32 global j-chunks
QC = 8                    # j-chunks per sigma quarter-slab
W1 = H + 1                # group width: [1 | h]
ALPHA, BETA, BN_EPS = 0.005, 5e-5, 1e-5

F32 = mybir.dt.float32
BF16 = mybir.dt.bfloat16
AF = mybir.ActivationFunctionType
ALU = mybir.AluOpType
XY = mybir.AxisListType.XY
AX = mybir.AxisListType.X

_CACHE = {}

# Prefer table sets so {Exp, Ln, Square} share one set: 5 loads total.
_orig_tables = None


def _patched_tables(arch):
    tabs = _orig_tables(arch)
    order = ["natural_log_exp_and_others", "sigmoid_and_others"]
    out = {k: tabs[k] for k in order if k in tabs}
    out.update({k: v for k, v in tabs.items() if k not in out})
    return out


def build_program():

    nc = bacc.Bacc("TRN2", target_bir_lowering=False, debug=False,
                   enable_asserts=False, num_devices=NC)

    def din(name, shape, dt):
        return nc.dram_tensor(name, shape, dt, kind="ExternalInput").ap()

    xT_aug = din("xT_aug", [NI + 1, B * G], F32)           # row 8 = ones
    xT_loc = din("xT_loc", [NI + 1, B * GL], F32)          # row 8 = ones
    w1T = din("w1T", [G, GL], BF16)
    w2T = din("w2T", [G, GL], BF16)
    W_aug = din("W_aug", [NI + 1, H], F32)
    We1_f = din("We1_f", [H + 1, 2], F32)
    We2_f = din("We2_f", [H + 1, 2], F32)
    We1_rep = din("We1_rep", [1, NCH * H], BF16)
    We2_rep = din("We2_rep", [1, NCH * H], BF16)
    Wn1a = din("Wn1a", [H + 1, NO], F32)                   # [0; W_n[:H]]
    Wn1b = din("Wn1b", [H + 1, NO], F32)                   # [W_n[H:]; b_n]
    Wm1a = din("Wm1a", [H + 1, NO], F32)
    Wm1b = din("Wm1b", [H + 1, NO], F32)
    Wn2a = din("Wn2a", [H + 1, NO], F32)
    Wn2b = din("Wn2b", [H + 1, NO], F32)
    Wm2a = din("Wm2a", [H + 1, NO], F32)
    Wm2b = din("Wm2b", [H + 1, NO], F32)
    bn_g_nat = din("bn_g_nat", [128, LCH], F32)
    bn_b_nat = din("bn_b_nat", [128, LCH], F32)
    bn_g_row = din("bn_g_row", [1, GL], F32)
    bn_b_row = din("bn_b_row", [1, GL], F32)

    out = nc.dram_tensor("out", [B * GL, NO], F32, kind="ExternalOutput").ap()
    out_r = out.rearrange("(b l p) f -> p b l f", b=B, l=LCH, p=128)

    with tile.TileContext(nc) as tc:
        with (
            tc.tile_pool(name="cp", bufs=1) as cp,
            tc.tile_pool(name="bp", bufs=1) as bp,
            tc.tile_pool(name="wp", bufs=1) as wp,
            tc.tile_pool(name="sp", bufs=2) as sp,
            tc.tile_pool(name="pp", bufs=1, space="PSUM") as pp,
            tc.tile_pool(name="dp", bufs=1, space="DRAM") as dp,
        ):
            # ---------- constants (small DMAs first: they gate compute) ----
            W_aug_sb = cp.tile([NI + 1, H], F32, name="W_aug_sb", tag="W_aug_sb")
            nc.sync.dma_start(W_aug_sb[:], W_aug[:])
            sm = {}
            for nm, ap_ in [("We1_rep", We1_rep), ("We2_rep", We2_rep),
                            ("We1_f", We1_f), ("We2_f", We2_f),
                            ("Wn1a", Wn1a), ("Wn1b", Wn1b),
                            ("Wm1a", Wm1a), ("Wm1b", Wm1b),
                            ("Wn2a", Wn2a), ("Wn2b", Wn2b),
                            ("Wm2a", Wm2a), ("Wm2b", Wm2b),
                            ("bn_g_nat", bn_g_nat), ("bn_b_nat", bn_b_nat),
                            ("bn_g_row", bn_g_row), ("bn_b_row", bn_b_row)]:
                t = cp.tile(list(ap_.shape), ap_.dtype, name=f"{nm}_sb",
                            tag=f"{nm}_sb")
                nc.sync.dma_start(t[:], ap_[:])
                sm[nm] = t
            ones_c = cp.tile([1, 128], F32, name="ones_c", tag="ones_c")
            nc.vector.memset(ones_c[:], 1.0)
            ones_cb = cp.tile([1, 128], BF16, name="ones_cb", tag="ones_cb")
            nc.vector.memset(ones_cb[:], 1.0)
            onesk = cp.tile([H, 1], F32, name="onesk", tag="onesk")
            nc.vector.memset(onesk[:], 1.0)
            xTl_sb = cp.tile([NI + 1, B * GL], F32, name="xTl_sb", tag="xTl_sb")
            nc.sync.dma_start(xTl_sb[:], xT_loc[:])

            # ---------- big resident tensors ----------
            h0n = bp.tile([128, B * NCH * W1], BF16, name="h0n", tag="h0n")
            h0l = bp.tile([H + 1, B * GL], F32, name="h0l", tag="h0l")
            nodes1T = bp.tile([H + 1, B * GL], F32, name="nodes1T", tag="nodes1T")
            nodes2T = bp.tile([H + 1, B * GL], F32, name="nodes2T", tag="nodes2T")
            hbnT_f = bp.tile([H + 1, B * GL], F32, name="hbnT_f", tag="hbnT_f")
            ghat = [bp.tile([128, NC * LCH * W1], BF16, name=f"ghat{b}",
                            tag=f"ghat{b}") for b in range(B)]
            nc.vector.memset(h0n[:], 1.0)
            nc.vector.memset(h0l[H:H + 1, :], 1.0)
            nc.vector.memset(nodes1T[H:H + 1, :], 1.0)
            nc.vector.memset(nodes2T[H:H + 1, :], 1.0)
            nc.vector.memset(hbnT_f[H:H + 1, :], 1.0)

            def elu(z_psum, out_ap, shape):
                p, f = shape
                tf = wp.tile([128, GL], F32, name="elu_t", tag="elu_t", bufs=3)
                t1 = tf[0:p, 0:f]
                nc.vector.tensor_scalar_min(t1, z_psum, 0.0)
                nc.scalar.activation(t1, t1, AF.Exp)
                nc.vector.tensor_scalar_add(t1, t1, -1.0)
                nc.vector.tensor_tensor(out_ap, z_psum, t1, op=ALU.max)

            # ---------- phase 1: h0 (natural layout, groups [1|h]) ----------
            h0n_v = h0n.rearrange("p (q e) -> p q e", e=W1)
            for kq in range(8):
                xq = wp.tile([NI + 1, 8 * 128], F32, name="xq", tag="xq", bufs=2)
                nc.sync.dma_start(xq[:], xT_aug[:, kq * 1024:(kq + 1) * 1024])
                ps = pp.tile([128, 8 * H], F32, name="ps_sm", tag="sm", bufs=4)
                for s in range(8):
                    nc.tensor.matmul(ps[:, s * H:(s + 1) * H],
                                     xq[:, s * 128:(s + 1) * 128],
                                     W_aug_sb[:], start=True, stop=True)
                elu(ps[:], h0n_v[:, kq * 8:(kq + 1) * 8, 1:W1], [128, 8 * H])
            for b in range(B):
                ps = pp.tile([H, GL], F32, name="ps_sm", tag="sm", bufs=4)
                nc.tensor.matmul(ps[:], W_aug_sb[:],
                                 xTl_sb[:, b * GL:(b + 1) * GL],
                                 start=True, stop=True)
                elu(ps[:], h0l[0:H, b * GL:(b + 1) * GL], [H, GL])

            # big edge-weight DMAs issued after the gating small ones
            w1T_sb = bp.tile([128, NCH * GL], BF16, name="w1T_sb", tag="w1T_sb")
            w2T_sb = bp.tile([128, NCH * GL], BF16, name="w2T_sb", tag="w2T_sb")
            w1T_r = w1T.rearrange("(k p) i -> p k i", p=128)
            w2T_r = w2T.rearrange("(k p) i -> p k i", p=128)
            for kq in range(4):
                nc.sync.dma_start(
                    w1T_sb[:, kq * QC * GL:(kq + 1) * QC * GL],
                    w1T_r[:, kq * QC:(kq + 1) * QC])

            gather_in = dp.tile([128, B * LCH * W1], BF16, name="gin",
                                tag="gin")
            gather_out = dp.tile([NC * 128, B * LCH * W1], BF16,
                                 addr_space="Shared", name="gout", tag="gout")

            # ---------- one message-passing block ----------
            def mp_block(blk, wT_sb, We_rep, We_f, Wna, Wnb, Wma, Wmb,
                         nat_of, hTl, nodesT, merge_dst):
                # s_src[p, col] = sum_f h_nat[p, g*33+1+f] * We_src[f]  (DVE)
                wrep = wp.tile([128, NCH * H], BF16, name="wrep", tag="wrep",
                               bufs=1)
                for c4 in range(NCH * H // 512):
                    ps_w = pp.tile([128, 512], F32, name="ps_w", tag="bc",
                                   bufs=2)
                    nc.tensor.matmul(ps_w[:], ones_cb[:],
                                     We_rep[:, c4 * 512:(c4 + 1) * 512],
                                     start=True, stop=True)
                    nc.vector.tensor_copy(wrep[:, c4 * 512:(c4 + 1) * 512],
                                          ps_w[:])
                wrep_v = wrep.rearrange("p (q f) -> p q f", f=H)
                ssrc = wp.tile([128, B * NCH], F32, name=f"ssrc{blk}",
                               tag=f"ssrc{blk}")
                for b in range(B):
                    h_nat, goff = nat_of(b)
                    h_nat_v = h_nat.rearrange("p (q e) -> p q e", e=W1)
                    ssx = wp.tile([128, NCH * H], BF16, name="ssx", tag="ssx",
                                  bufs=2)
                    ssx_v = ssx.rearrange("p (q f) -> p q f", f=H)
                    nc.vector.tensor_tensor(
                        ssx_v, h_nat_v[:, goff:goff + NCH, 1:W1], wrep_v,
                        op=ALU.mult)
                    nc.vector.reduce_sum(ssrc[:, b * NCH:(b + 1) * NCH],
                                         ssx_v, axis=AX)
                accs = []
                for b in range(B):
                    h_nat, goff = nat_of(b)
                    h_nat_v = h_nat.rearrange("p (q e) -> p q e", e=W1)
                    ps_d = pp.tile([1, GL], F32, name="ps_d", tag="sm", bufs=4)
                    nc.tensor.matmul(ps_d[:], We_f[:, 1:2],
                                     hTl[:, b * GL:(b + 1) * GL],
                                     start=True, stop=True)
                    sd_row = wp.tile([1, GL], F32, name="sd_row", tag="sd_row",
                                     bufs=2)
                    nc.vector.tensor_copy(sd_row[:], ps_d[:])
                    ps_bc = pp.tile([128, GL], F32, name="ps_bc", tag="bc",
                                    bufs=2)
                    nc.tensor.matmul(ps_bc[:], ones_c[:], sd_row[:],
                                     start=True, stop=True)
                    sdb = wp.tile([128, GL], F32, name="sdb", tag="sdb", bufs=2)
                    nc.vector.tensor_copy(sdb[:], ps_bc[:])

                    ps_acc = pp.tile([W1, GL], F32, name="ps_acc", tag="acc",
                                     bufs=2)
                    for qq in range(NCH // QC):
                        sig = sp.tile([128, QC * GL], BF16, name="sig",
                                      tag="sig", bufs=2)
                        for k8 in range(QC):
                            k = qq * QC + k8
                            nc.scalar.activation(
                                sig[:, k8 * GL:(k8 + 1) * GL], sdb[:],
                                AF.Sigmoid,
                                bias=ssrc[:, b * NCH + k:b * NCH + k + 1])
                        for hh in range(QC // 4):
                            sl = slice(hh * 4 * GL, (hh + 1) * 4 * GL)
                            wsl = slice((qq * QC + hh * 4) * GL,
                                        (qq * QC + hh * 4 + 4) * GL)
                            nc.vector.tensor_tensor(sig[:, sl], sig[:, sl],
                                                    wT_sb[:, wsl], op=ALU.mult)
                        for k8 in range(QC):
                            k = qq * QC + k8
                            nc.tensor.matmul(
                                ps_acc[:], h_nat_v[:, goff + k, :],
                                sig[:, k8 * GL:(k8 + 1) * GL],
                                start=(k == 0), stop=(k == NCH - 1))
                    accs.append(ps_acc)
                for b in range(B):
                    ps_acc = accs[b]
                    # rows: 0 = rowsum, 1..32 = recv_srcT
                    rfull = wp.tile([H + 1, GL], F32, name="rfull", tag="rfull",
                                    bufs=2)
                    nc.vector.tensor_copy(rfull[:], ps_acc[:])
                    ps_rb = pp.tile([H, GL], F32, name="ps_rb", tag="bc", bufs=2)
                    nc.tensor.matmul(ps_rb[:], ones_c[:, 0:H], rfull[0:1, :],
                                     start=True, stop=True)
                    hdT = wp.tile([H + 1, GL], F32, name="hdT", tag="hdT",
                                  bufs=2)
                    nc.vector.tensor_tensor(hdT[0:H, :],
                                            hTl[0:H, b * GL:(b + 1) * GL],
                                            ps_rb[:], op=ALU.mult)
                    nc.vector.memset(hdT[H:H + 1, :], 1.0)
                    ps_n = pp.tile([H, GL], F32, name="ps_n", tag="sm", bufs=4)
                    nc.tensor.matmul(ps_n[:], Wna[:], rfull[:],
                                     start=True, stop=False)
                    nc.tensor.matmul(ps_n[:], Wnb[:], hdT[:],
                                     start=False, stop=True)
                    elu(ps_n[:], nodesT[0:H, b * GL:(b + 1) * GL], [H, GL])
                    ps_m = pp.tile([128, LCH * NO], F32, name="ps_m", tag="sm",
                                   bufs=4)
                    for l in range(LCH):
                        c0 = b * GL + l * 128
                        nc.tensor.matmul(ps_m[:, l * NO:(l + 1) * NO],
                                         nodesT[:, c0:c0 + 128],
                                         Wma[:], start=True, stop=False)
                        nc.tensor.matmul(ps_m[:, l * NO:(l + 1) * NO],
                                         hTl[:, c0:c0 + 128],
                                         Wmb[:], start=False, stop=True)
                    merge_dst(b, ps_m)

            # ---------- block 1 ----------
            h1n = wp.tile([128, B * LCH * NO], F32, name="h1n", tag="h1n")

            def merge1_dst(b, ps_m):
                c0 = b * LCH * NO
                elu(ps_m[:], h1n[:, c0:c0 + LCH * NO], [128, LCH * NO])

            mp_block(1, w1T_sb, sm["We1_rep"], sm["We1_f"],
                     sm["Wn1a"], sm["Wn1b"], sm["Wm1a"], sm["Wm1b"],
                     lambda b: (h0n, b * NCH), h0l, nodes1T, merge1_dst)
            for kq in range(4):
                nc.sync.dma_start(
                    w2T_sb[:, kq * QC * GL:(kq + 1) * QC * GL],
                    w2T_r[:, kq * QC:(kq + 1) * QC])

            h1T = wp.tile([H, B * GL], F32, name="h1T", tag="h1T")
            for b in range(B):
                ps = pp.tile([H, GL], F32, name="ps_sm2", tag="sm", bufs=4)
                nc.tensor.matmul(ps[:], sm["Wm1a"][:],
                                 nodes1T[:, b * GL:(b + 1) * GL],
                                 start=True, stop=False)
                nc.tensor.matmul(ps[:], sm["Wm1b"][:],
                                 h0l[:, b * GL:(b + 1) * GL],
                                 start=False, stop=True)
                elu(ps[:], h1T[:, b * GL:(b + 1) * GL], [H, GL])

            # ---------- BatchNorm (fully local) ----------
            stat = wp.tile([128, 6 * LCH], F32, name="stat", tag="stat")
            mu_n, var_n = stat[:, 0:LCH], stat[:, LCH:2 * LCH]
            scl_n, shf_n = stat[:, 2 * LCH:3 * LCH], stat[:, 3 * LCH:4 * LCH]
            t_n, t2_n = stat[:, 4 * LCH:5 * LCH], stat[:, 5 * LCH:6 * LCH]
            sq_n = wp.tile([128, B * LCH * NO], F32, name="sq_n", tag="sq_n")
            nc.scalar.activation(sq_n[:], h1n[:], AF.Square)
            h1n_r = h1n.rearrange("p (b l f) -> p b l f", b=B, l=LCH)
            sq_r = sq_n.rearrange("p (b l f) -> p b l f", b=B, l=LCH)
            for l in range(LCH):
                nc.vector.reduce_sum(mu_n[:, l:l + 1], h1n_r[:, :, l, :], axis=XY)
                nc.vector.reduce_sum(var_n[:, l:l + 1], sq_r[:, :, l, :], axis=XY)
            nc.vector.tensor_scalar_mul(mu_n, mu_n, 1.0 / (B * NO))
            nc.vector.tensor_scalar_mul(var_n, var_n, 1.0 / (B * NO))
            nc.vector.tensor_tensor(t_n, mu_n, mu_n, op=ALU.mult)
            nc.vector.tensor_tensor(var_n, var_n, t_n, op=ALU.subtract)
            nc.vector.tensor_scalar_add(t_n, var_n, BN_EPS)
            nc.scalar.activation(t_n, t_n, AF.Ln)
            nc.scalar.activation(t_n, t_n, AF.Exp, scale=-0.5)
            nc.vector.tensor_tensor(scl_n, t_n, sm["bn_g_nat"][:], op=ALU.mult)
            nc.vector.tensor_tensor(t2_n, mu_n, scl_n, op=ALU.mult)
            nc.vector.tensor_tensor(shf_n, sm["bn_b_nat"][:], t2_n,
                                    op=ALU.subtract)
            # normalized h, natural groups [1|h]; per-b gather as soon as ready
            hbn_n = wp.tile([128, B * LCH * W1], BF16, name="hbn_n",
                            tag="hbn_n")
            nc.vector.memset(hbn_n[:], 1.0)
            for b in range(B):
                for l in range(LCH):
                    q = b * LCH + l
                    nc.vector.tensor_scalar(
                        hbn_n[:, q * W1 + 1:(q + 1) * W1],
                        h1n[:, (b * LCH + l) * NO:(b * LCH + l + 1) * NO],
                        scl_n[:, l:l + 1], shf_n[:, l:l + 1],
                        op0=ALU.mult, op1=ALU.add)
            nc.sync.dma_start(gather_in[:], hbn_n[:])
            nc.gpsimd.collective_compute(
                "AllGather", ALU.bypass, replica_groups=[list(range(NC))],
                ins=[gather_in.opt()], outs=[gather_out.opt()])
            for b in range(B):
                for c in range(NC):
                    nc.sync.dma_start(
                        ghat[b][:, c * LCH * W1:(c + 1) * LCH * W1],
                        gather_out[c * 128:(c + 1) * 128,
                                   b * LCH * W1:(b + 1) * LCH * W1])

            # row-layout stats for the feature-major copy
            rowb = wp.tile([1, 4 * GL], F32, name="rowb", tag="rowb")
            mu_r, var_r = rowb[:, 0:GL], rowb[:, GL:2 * GL]
            scl_r, shf_r = rowb[:, 2 * GL:3 * GL], rowb[:, 3 * GL:4 * GL]
            t_r, t2_r = scl_r, shf_r
            sqT = wp.tile([H, B * GL], F32, name="sqT", tag="sqT")
            nc.scalar.activation(sqT[:], h1T[:], AF.Square)
            ps_r0 = pp.tile([1, GL], F32, name="ps_r0", tag="sm", bufs=4)
            for b in range(B):
                nc.tensor.matmul(ps_r0[:], onesk[:],
                                 h1T[:, b * GL:(b + 1) * GL],
                                 start=(b == 0), stop=(b == B - 1))
            ps_r1 = pp.tile([1, GL], F32, name="ps_r1", tag="sm", bufs=4)
            for b in range(B):
                nc.tensor.matmul(ps_r1[:], onesk[:],
                                 sqT[:, b * GL:(b + 1) * GL],
                                 start=(b == 0), stop=(b == B - 1))
            nc.vector.tensor_scalar_mul(mu_r, ps_r0[:], 1.0 / (B * NO))
            nc.vector.tensor_scalar_mul(var_r, ps_r1[:], 1.0 / (B * NO))
            nc.vector.tensor_tensor(t_r, mu_r, mu_r, op=ALU.mult)
            nc.vector.tensor_tensor(var_r, var_r, t_r, op=ALU.subtract)
            nc.vector.tensor_scalar_add(t_r, var_r, BN_EPS)
            nc.scalar.activation(t_r, t_r, AF.Ln)
            nc.scalar.activation(t_r, t_r, AF.Exp, scale=-0.5)
            nc.vector.tensor_tensor(scl_r, t_r, sm["bn_g_row"][:], op=ALU.mult)
            nc.vector.tensor_tensor(t2_r, mu_r, scl_r, op=ALU.mult)
            nc.vector.tensor_tensor(shf_r, sm["bn_b_row"][:], t2_r,
                                    op=ALU.subtract)
            ps_sc = pp.tile([H, GL], F32, name="ps_sc", tag="bc", bufs=2)
            nc.tensor.matmul(ps_sc[:], ones_c[:, 0:H], scl_r, start=True,
                             stop=True)
            ps_sh = pp.tile([H, GL], F32, name="ps_sh", tag="bc", bufs=2)
            nc.tensor.matmul(ps_sh[:], ones_c[:, 0:H], shf_r, start=True,
                             stop=True)
            for b in range(B):
                sl = slice(b * GL, (b + 1) * GL)
                nc.vector.tensor_tensor(hbnT_f[0:H, sl], h1T[:, sl], ps_sc[:],
                                        op=ALU.mult)
                nc.vector.tensor_tensor(hbnT_f[0:H, sl], hbnT_f[0:H, sl],
                                        ps_sh[:], op=ALU.add)

            # ---------- block 2 ----------
            out_n = wp.tile([128, B * LCH * NO], F32, name="out_n", tag="out_n")

            def merge2_dst(b, ps_m):
                c0 = b * LCH * NO
                elu(ps_m[:], out_n[:, c0:c0 + LCH * NO], [128, LCH * NO])

            mp_block(2, w2T_sb, sm["We2_rep"], sm["We2_f"],
                     sm["Wn2a"], sm["Wn2b"], sm["Wm2a"], sm["Wm2b"],
                     lambda b: (ghat[b], 0), hbnT_f, nodes2T, merge2_dst)

            nc.sync.dma_start(out_r, out_n[:])

    nc.compile()
    return nc


def _prep_inputs(x, edges1, edges2, W_infer, b_infer, W_e1, b_e1, W_e2, b_e2,
                 W_n1, b_n1, W_n2, b_n2, W_m1, b_m1, W_m2, b_m2,
                 bn_gamma, bn_beta):
    f32 = np.float32
    bf16 = ml_dtypes.bfloat16
    xT = np.asarray(x, f32).transpose(2, 0, 1).reshape(NI, B * G)
    xT_aug = np.concatenate([xT, np.ones((1, B * G), f32)], axis=0)
    w1 = (ALPHA + (1.0 - ALPHA) * np.asarray(edges1, f32)).astype(bf16)
    w2 = (BETA + (1.0 - BETA) * np.asarray(edges2, f32)).astype(bf16)

    def wecat(W_e, b_e):
        c0 = np.concatenate([np.asarray(W_e, f32)[:H, 0], [0.0]]).astype(f32)
        c1 = np.concatenate([np.asarray(W_e, f32)[H:, 0],
                             [np.asarray(b_e, f32)[0]]]).astype(f32)
        return np.stack([c0, c1], axis=1)

    We1 = wecat(W_e1, b_e1)
    We2 = wecat(W_e2, b_e2)
    z = np.zeros((1, NO), f32)

    def stk(Wpart, brow):
        return np.concatenate([np.asarray(Wpart, f32), brow], 0)

    com = dict(
        xT_aug=xT_aug,
        W_aug=np.concatenate([np.asarray(W_infer, f32),
                              np.asarray(b_infer, f32)[None, :]], 0),
        We1_rep=np.tile(We1[:H, 0], NCH)[None, :].astype(bf16),
        We2_rep=np.tile(We2[:H, 0], NCH)[None, :].astype(bf16),
        We1_f=We1, We2_f=We2,
        Wn1a=np.concatenate([z, np.asarray(W_n1, f32)[:H]], 0),
        Wn1b=stk(np.asarray(W_n1, f32)[H:], np.asarray(b_n1, f32)[None, :]),
        Wm1a=stk(np.asarray(W_m1, f32)[:H], np.asarray(b_m1, f32)[None, :]),
        Wm1b=stk(np.asarray(W_m1, f32)[H:], z),
        Wn2a=np.concatenate([z, np.asarray(W_n2, f32)[:H]], 0),
        Wn2b=stk(np.asarray(W_n2, f32)[H:], np.asarray(b_n2, f32)[None, :]),
        Wm2a=stk(np.asarray(W_m2, f32)[:H], np.asarray(b_m2, f32)[None, :]),
        Wm2b=stk(np.asarray(W_m2, f32)[H:], z),
    )
    in_maps = []
    for c in range(NC):
        sl = slice(c * GL, (c + 1) * GL)
        xl = np.asarray(x, f32)[:, sl, :].transpose(2, 0, 1).reshape(NI, B * GL)
        m = dict(com)
        m["xT_loc"] = np.concatenate([xl, np.ones((1, B * GL), f32)], 0)
        m["w1T"] = np.ascontiguousarray(w1[sl, :].T)
        m["w2T"] = np.ascontiguousarray(w2[sl, :].T)
        g = np.asarray(bn_gamma, f32)[sl]
        b_ = np.asarray(bn_beta, f32)[sl]
        m["bn_g_nat"] = np.ascontiguousarray(g.reshape(LCH, 128).T)
        m["bn_b_nat"] = np.ascontiguousarray(b_.reshape(LCH, 128).T)
        m["bn_g_row"] = np.ascontiguousarray(g[None, :])
        m["bn_b_row"] = np.ascontiguousarray(b_[None, :])
        in_maps.append(m)
    return in_maps


def kernel(**inputs):
    if "nc" not in _CACHE:
        _CACHE["nc"] = build_program()
    nc = _CACHE["nc"]
    in_maps = _prep_inputs(**inputs)
    res = run_bass_kernel_spmd(nc, in_maps, list(range(NC)))
    parts = [res.results[c]["out"].reshape(B, GL, NO) for c in range(NC)]
    return np.concatenate(parts, axis=1).astype(np.float32)



# revision 16
# speedup vs baseline: 1.2707x; 1.2707x over previous
"""Trainium2 Bass kernel for gnn_message_passing (nn_BFR_28089086116615).

v2 design (transposed-primary):
- Receiver axis i sharded (G=4096 -> 8 cores x 512). Edge matrices are
  host-transposed and gated in bf16: wT[j, i].
- Phase 1 computes h0^T = elu(W^T x) in a column-folded layout (4 row-groups
  of 32 features x 2048 cols) with 16 big N=512 bf16 matmuls, then 16 PE
  transposes ([128,128] blocks) rebuild the natural [gene-partition | 1|h]
  groups for the contraction lhsT. Per-core gene permutation (local genes
  first) keeps the local receiver slice at a fixed, core-independent offset.
- All small matmuls run in bf16 (1 cyc/row on PE vs 4 for fp32).
- Partition broadcasts (sdb row, rowsum, BN scale/shift) run on GpSimd
  instead of PE+DVE.
- BatchNorm is per-gene -> fully local, computed in row layout only.
- sigma^T is produced chunk-wise by ACT (sigmoid, per-partition bias =
  ssrc[j-chunk]), gated on DVE in bf16, contracted on PE with stationary
  [1|h] groups so the receiver rowsum lands in psum row 0.
- ACT work is batched by table set (sigmoid vs exp) to minimize the 1.3us
  activation table reloads.
- One AllGather of the normalized h (natural [1|h] groups) between blocks;
  scattered to SBUF with a single strided DMA.
"""
import sys
sys.path.insert(0, "/opt/trn_rl_repo")
import numpy as np
import ml_dtypes

import concourse.bass as bass
import concourse.bacc as bacc
import concourse.mybir as mybir
import concourse.tile as tile
from concourse import masks
from concourse.bass_utils import run_bass_kernel_spmd

NC = 8
B, G, NI, H, NO = 2, 4096, 8, 32, 32
GL = G // NC              # 512 local receivers per core
LCH = GL // 128           # 4 local chunks
NCH = G // 128            # 32 global j-chunks
W1 = H + 1                # group width: [1 | h]
ALPHA, BETA, BN_EPS = 0.005, 5e-5, 1e-5

F32 = mybir.dt.float32
F32R = mybir.dt.float32r
BF16 = mybir.dt.bfloat16
AF = mybir.ActivationFunctionType
ALU = mybir.AluOpType
XY = mybir.AxisListType.XY
AX = mybir.AxisListType.X

# pack_b column layout (bf16)
PK_WREP1 = 0                      # [128, NCH*W1] wrep1e (b_e1 in slot 0)
PK_WREP2 = NCH * W1               # [128, NCH*W1]
PK_BASE = 2 * NCH * W1            # 2112
PK_WAUG = PK_BASE                 # [9, 32]
PK_WE1D = PK_BASE + 32            # [32, 1]
PK_WE2D = PK_BASE + 33            # [32, 1]
PK_W8 = PK_BASE + 34              # 8 blocks of 32: Wn1a Wn1b Wm1a Wm1b Wn2a Wn2b Wm2a Wm2b
PK_W = PK_W8 + 8 * 32             # total width 2402
# pack_f column layout (fp32): 8 weight blocks, then bn_g/bn_b rows
PF_W8 = 0
PF_BNG = 256
PF_BNB = PF_BNG + GL
PF_ONER = PF_BNB + GL             # [1, GL] ones row
PF_ONEC = PF_ONER + GL            # [33, 1] ones column
PF_W = PF_ONEC + 1

_CACHE = {}


def build_program():
    nc = bacc.Bacc("TRN2", target_bir_lowering=False, debug=False,
                   enable_asserts=False, num_devices=NC)

    def din(name, shape, dt):
        return nc.dram_tensor(name, shape, dt, kind="ExternalInput").ap()

    xT_b = din("xT_b", [NI + 1, B * G], BF16)
    w1T = din("w1T", [G, GL], BF16)
    w2T = din("w2T", [G, GL], BF16)
    pack_b = din("pack_b", [128, PK_W], BF16)
    pack_f = din("pack_f", [33, PF_W], F32R)

    out = nc.dram_tensor("out", [B * GL, NO], F32, kind="ExternalOutput").ap()
    out_r = out.rearrange("(b l p) f -> p b l f", b=B, l=LCH, p=128)

    with tile.TileContext(nc) as tc:
        with (
            tc.tile_pool(name="cp", bufs=1) as cp,
            tc.tile_pool(name="bp", bufs=1) as bp,
            tc.tile_pool(name="wp", bufs=1) as wp,
            tc.tile_pool(name="sp", bufs=2) as sp,
            tc.tile_pool(name="pp", bufs=1, space="PSUM") as pp,
            tc.tile_pool(name="dp", bufs=1, space="DRAM") as dp,
        ):
            # ---------------- input DMAs (small gating ones first) --------
            pack_f_sb = cp.tile([33, PF_W], F32R, name="pack_f_sb",
                                tag="pack_f_sb")
            nc.sync.dma_start(pack_f_sb[:], pack_f[:])
            pack_sb = cp.tile([128, PK_W], BF16, name="pack_sb", tag="pack_sb")
            nc.sync.dma_start(pack_sb[:], pack_b[:])
            xq = cp.tile([NI + 1, B * G], BF16, name="xq", tag="xq")
            for s in range(4):
                nc.sync.dma_start(xq[:, s * 2048:(s + 1) * 2048],
                                  xT_b[:, s * 2048:(s + 1) * 2048])
            w1T_sb = bp.tile([128, NCH * GL], BF16, name="w1T_sb", tag="w1T_sb")
            w2T_sb = bp.tile([128, NCH * GL], BF16, name="w2T_sb", tag="w2T_sb")
            w1T_r = w1T.rearrange("(k p) i -> p k i", p=128)
            w2T_r = w2T.rearrange("(k p) i -> p k i", p=128)
            for kq in range(4):
                nc.sync.dma_start(
                    w1T_sb[:, kq * 8 * GL:(kq + 1) * 8 * GL],
                    w1T_r[:, kq * 8:(kq + 1) * 8])

            # views into the const pack
            wrep1_v = pack_sb[:, PK_WREP1:PK_WREP1 + NCH * W1].rearrange(
                "p (k e) -> p k e", e=W1)
            wrep2_v = pack_sb[:, PK_WREP2:PK_WREP2 + NCH * W1].rearrange(
                "p (k e) -> p k e", e=W1)
            W_aug = pack_sb[0:NI + 1, PK_WAUG:PK_WAUG + 32]
            We1_d = pack_sb[0:H, PK_WE1D:PK_WE1D + 1]
            We2_d = pack_sb[0:H, PK_WE2D:PK_WE2D + 1]

            def wblk(i, p):
                return pack_f_sb[0:p, PF_W8 + i * 32:PF_W8 + (i + 1) * 32]
            Wn1a, Wn1b = wblk(0, 33), wblk(1, 33)
            Wm1a, Wm1b = wblk(2, 33), wblk(3, 32)
            Wn2a, Wn2b = wblk(4, 33), wblk(5, 33)
            Wm2a, Wm2b = wblk(6, 33), wblk(7, 32)
            Wm1b_h = pack_sb[0:32, PK_W8 + 3 * 32:PK_W8 + 4 * 32]
            bn_g = pack_f_sb[0:1, PF_BNG:PF_BNG + GL].bitcast(F32)
            bn_b = pack_f_sb[0:1, PF_BNB:PF_BNB + GL].bitcast(F32)

            # ---------------- constants / identities ----------------------
            onesk_f = pack_f_sb[0:H, PF_ONEC:PF_ONEC + 1]
            oner_f = pack_f_sb[0:1, PF_ONER:PF_ONER + GL]
            id_bf = cp.tile([128, 128], BF16, name="id_bf", tag="id_bf")
            masks.make_identity(nc, id_bf[:])
            id_f = cp.tile([32, 32], F32, name="id_f", tag="id_f")
            masks.make_identity(nc, id_f[:])

            # ---------------- big resident tensors ------------------------
            h0T = bp.tile([128, 2048], BF16, name="h0T", tag="h0T")
            h0n = bp.tile([128, B * NCH * W1], BF16, name="h0n", tag="h0n")
            nc.vector.memset(h0n[:], 1.0)
            ghat = bp.tile([128, B * NCH * W1], BF16, name="ghat", tag="ghat")
            h1T = bp.tile([H, B * GL], F32R, name="h1T", tag="h1T")
            hbnT = bp.tile([H, B * GL], F32R, name="hbnT", tag="hbnT")
            hbn_b = bp.tile([H, B * GL], BF16, name="hbn_b", tag="hbn_b")
            hbn_n = wp.tile([128, B * LCH * W1], BF16, name="hbn_n",
                            tag="hbn_n")
            nc.vector.memset(hbn_n[:], 1.0)

            gather_in = dp.tile([128, B * LCH * W1], BF16, name="gin",
                                tag="gin")
            gather_out = dp.tile([NC * 128, B * LCH * W1], BF16,
                                 addr_space="Shared", name="gout", tag="gout")

            hdTs, nodess = [], []
            for i in range(2):
                hdTx = wp.tile([W1, GL], F32R, name=f"hdT{i}", tag=f"hdT{i}")
                nc.vector.tensor_copy(hdTx[H:W1, :], oner_f)
                hdTs.append(hdTx)
                nodesx = wp.tile([W1, GL], F32R, name=f"nodes{i}",
                                 tag=f"nodes{i}")
                nc.vector.tensor_copy(nodesx[H:W1, :], oner_f)
                nodess.append(nodesx)

            def elu(z_psum, out_ap, p, f):
                t = wp.tile([128, 512], F32, name="elu_t", tag="elu_t",
                            bufs=3)[0:p, 0:f]
                nc.vector.tensor_scalar_min(t, z_psum, 0.0)
                nc.scalar.activation(t, t, AF.Exp)
                nc.vector.tensor_scalar_add(t, t, -1.0)
                nc.vector.tensor_tensor(out_ap, z_psum, t, op=ALU.max)

            # ---------------- phase 1: h0T fold + natural -----------------
            # fold[32a+f, 512s+j] = h0T[f, (4s+a)*512+j]
            for s in range(4):
                ps = pp.tile([128, 512], F32, name="ps_ph0", tag="ph0", bufs=2)
                for a in range(4):
                    q = 4 * s + a
                    nc.tensor.matmul(ps[32 * a:32 * a + 32, :], W_aug,
                                     xq[:, q * 512:(q + 1) * 512],
                                     start=True, stop=True,
                                     tile_position=(0, 32 * a))
                elu(ps[:], h0T[:, s * 512:(s + 1) * 512], 128, 512)
            # natural layout via PE transposes: group m = 16u + t1 + 4a
            h0n_g = h0n.rearrange("p (u a t e) -> p u a t e", u=4, a=4, t=4)
            for u in range(4):
                trp = pp.tile([128, 512], BF16, name="ps_tr", tag="tr", bufs=1)
                for t1 in range(4):
                    nc.tensor.transpose(
                        trp[:, t1 * 128:(t1 + 1) * 128],
                        h0T[:, (4 * u + t1) * 128:(4 * u + t1 + 1) * 128],
                        id_bf[:])
                trp_v = trp.rearrange("p (t a e) -> p t a e", t=4, a=4)
                nc.vector.tensor_copy(
                    h0n_g[:, u, :, :, 1:W1].transpose([0, 2, 1, 3]), trp_v)

            # local receiver features: h0l(b) = fold[0:32, 1024b : 1024b+512]
            def h0l(b):
                return h0T[0:H, 1024 * b:1024 * b + 512]

            h0n_v = h0n.rearrange("p (g e) -> p g e", e=W1)
            ghat_v = ghat.rearrange("p (g e) -> p g e", e=W1)

            # issue w2T loads behind w1T
            for kq in range(4):
                nc.sync.dma_start(
                    w2T_sb[:, kq * 8 * GL:(kq + 1) * 8 * GL],
                    w2T_r[:, kq * 8:(kq + 1) * 8])

            # ---------------- shared mp-block pieces ----------------------
            def ssrc_calc(blk, nat_v, wrep_v):
                ssrc = wp.tile([128, B * NCH], F32, name=f"ssrc{blk}",
                               tag=f"ssrc{blk}")
                for b in range(B):
                    ssx = wp.tile([128, NCH * W1], BF16, name="ssx", tag="ssx",
                                  bufs=2)
                    ssx_v = ssx.rearrange("p (k e) -> p k e", e=W1)
                    nc.vector.tensor_tensor(
                        ssx_v, nat_v[:, b * NCH:(b + 1) * NCH, :], wrep_v,
                        op=ALU.mult)
                    nc.vector.reduce_sum(ssrc[:, b * NCH:(b + 1) * NCH],
                                         ssx_v, axis=AX)
                return ssrc

            def sdb_calc(blk, We_d, hTl_of):
                sd_row = wp.tile([1, B * GL], F32, name=f"sd{blk}",
                                 tag=f"sd{blk}")
                for b in range(B):
                    ps_d = pp.tile([1, 512], F32, name="ps_d", tag="sm",
                                   bufs=2)
                    nc.tensor.matmul(ps_d[:], We_d, hTl_of(b),
                                     start=True, stop=True)
                    nc.vector.tensor_copy(sd_row[:, b * GL:(b + 1) * GL],
                                          ps_d[:])
                sdb = wp.tile([128, B * GL], F32, name=f"sdb{blk}",
                              tag=f"sdb{blk}")
                nc.gpsimd.partition_broadcast(sdb[:], sd_row[:])
                return sdb

            def contraction(b, wT_sb, sdb, ssrc, nat_v):
                ps_acc = pp.tile([W1, GL], F32, name="ps_acc", tag="acc",
                                 bufs=2)
                for qq in range(4):
                    sig = sp.tile([128, 8 * GL], BF16, name="sig", tag="sig",
                                  bufs=2)
                    for k8 in range(8):
                        k = qq * 8 + k8
                        nc.scalar.activation(
                            sig[:, k8 * GL:(k8 + 1) * GL],
                            sdb[:, b * GL:(b + 1) * GL], AF.Sigmoid,
                            bias=ssrc[:, b * NCH + k:b * NCH + k + 1])
                    for hh in range(2):
                        sl = slice(hh * 4 * GL, (hh + 1) * 4 * GL)
                        wsl = slice((qq * 8 + hh * 4) * GL,
                                    (qq * 8 + hh * 4 + 4) * GL)
                        nc.vector.tensor_tensor(sig[:, sl], sig[:, sl],
                                                wT_sb[:, wsl], op=ALU.mult)
                    for k8 in range(8):
                        k = qq * 8 + k8
                        nc.tensor.matmul(
                            ps_acc[:], nat_v[:, b * NCH + k, :],
                            sig[:, k8 * GL:(k8 + 1) * GL],
                            start=(k == 0), stop=(k == NCH - 1))
                return ps_acc

            def post_acc(b, ps_acc, hTl_b, hTl_mrg, Wna, Wnb, Wma, Wmb,
                         mrg_out, mrg_p):
                rfull = wp.tile([W1, GL], F32R, name="rfull", tag="rfull",
                                bufs=2)
                nc.vector.tensor_copy(rfull[:], ps_acc[:])
                rb = wp.tile([H, GL], F32R, name="rb", tag="rb", bufs=2)
                nc.gpsimd.partition_broadcast(rb[:], rfull[0:1, :])
                hdT = hdTs[b]
                nc.vector.tensor_tensor(hdT[0:H, :], hTl_b, rb.bitcast(F32),
                                        op=ALU.mult)
                ps_n = pp.tile([H, GL], F32, name="ps_n", tag="sm", bufs=2)
                nc.tensor.matmul(ps_n[:], Wna, rfull[:],
                                 start=True, stop=False)
                nc.tensor.matmul(ps_n[:], Wnb, hdT[:],
                                 start=False, stop=True)
                nodes = nodess[b]
                elu(ps_n[:], nodes[0:H, :], H, GL)
                ps_m = pp.tile([H, GL], F32, name="ps_m", tag="sm", bufs=2)
                nc.tensor.matmul(ps_m[:], Wma, nodes[:],
                                 start=True, stop=False)
                hT_m, Wmb_m = hTl_mrg
                nc.tensor.matmul(ps_m[:], Wmb_m, hT_m, start=False, stop=True)
                elu(ps_m[:], mrg_out, mrg_p, GL)

            # ---------------- block 1 -------------------------------------
            ssrc1 = ssrc_calc(1, h0n_v, wrep1_v)
            sdb1 = sdb_calc(1, We1_d, h0l)
            accs = [contraction(b, w1T_sb, sdb1, ssrc1, h0n_v)
                    for b in range(B)]
            for b in range(B):
                post_acc(b, accs[b], h0l(b), (h0l(b), Wm1b_h),
                         Wn1a, Wn1b, Wm1a, Wm1b,
                         h1T[:, b * GL:(b + 1) * GL], H)

            # ---------------- BatchNorm (row layout, fully local) ---------
            sqT = wp.tile([H, B * GL], F32R, name="sqT", tag="sqT")
            nc.vector.tensor_tensor(sqT[:], h1T.bitcast(F32), h1T.bitcast(F32),
                                    op=ALU.mult)
            ps_r0 = pp.tile([1, GL], F32, name="ps_r0", tag="sm", bufs=2)
            for b in range(B):
                nc.tensor.matmul(ps_r0[:], onesk_f[:],
                                 h1T[:, b * GL:(b + 1) * GL],
                                 start=(b == 0), stop=(b == B - 1))
            ps_r1 = pp.tile([1, GL], F32, name="ps_r1", tag="sm", bufs=2)
            for b in range(B):
                nc.tensor.matmul(ps_r1[:], onesk_f[:],
                                 sqT[:, b * GL:(b + 1) * GL],
                                 start=(b == 0), stop=(b == B - 1))
            rowb = wp.tile([1, 4 * GL], F32, name="rowb", tag="rowb")
            mu_r, var_r = rowb[:, 0:GL], rowb[:, GL:2 * GL]
            scl_r, shf_r = rowb[:, 2 * GL:3 * GL], rowb[:, 3 * GL:4 * GL]
            nc.vector.tensor_scalar_mul(mu_r, ps_r0[:], 1.0 / (B * NO))
            nc.vector.tensor_scalar_mul(var_r, ps_r1[:], 1.0 / (B * NO))
            nc.vector.tensor_tensor(scl_r, mu_r, mu_r, op=ALU.mult)
            nc.vector.tensor_tensor(var_r, var_r, scl_r, op=ALU.subtract)
            nc.vector.tensor_scalar_add(scl_r, var_r, BN_EPS)
            nc.scalar.activation(scl_r, scl_r, AF.Ln)
            nc.scalar.activation(scl_r, scl_r, AF.Exp, scale=-0.5)
            nc.vector.tensor_tensor(scl_r, scl_r, bn_g, op=ALU.mult)
            nc.vector.tensor_tensor(shf_r, mu_r, scl_r, op=ALU.mult)
            nc.vector.tensor_tensor(shf_r, bn_b, shf_r, op=ALU.subtract)
            ssb = wp.tile([H, 2 * GL], F32, name="ssb", tag="ssb")
            nc.gpsimd.partition_broadcast(ssb[:], rowb[:, 2 * GL:4 * GL])
            for b in range(B):
                sl = slice(b * GL, (b + 1) * GL)
                nc.vector.tensor_tensor(hbnT[:, sl],
                                        h1T[:, sl].bitcast(F32), ssb[:, 0:GL],
                                        op=ALU.mult)
                nc.vector.tensor_tensor(hbnT[:, sl], hbnT[:, sl].bitcast(F32),
                                        ssb[:, GL:2 * GL], op=ALU.add)
            nc.vector.tensor_copy(hbn_b[:], hbnT.bitcast(F32))

            # block-2 sdb can go before the gather (local only)
            sdb2 = sdb_calc(2, We2_d,
                            lambda b: hbn_b[:, b * GL:(b + 1) * GL])

            # ---------------- gather of normalized h ----------------------
            trh = pp.tile([128, 512], BF16, name="ps_trh", tag="tr", bufs=1)
            for j in range(B * LCH):
                nc.tensor.transpose(trh[:, j * 32:(j + 1) * 32],
                                    hbn_b[:, j * 128:(j + 1) * 128],
                                    id_bf[0:H, 0:H])
            hbn_g = hbn_n.rearrange("p (g e) -> p g e", e=W1)
            trh_v = trh[:, 0:B * LCH * H].rearrange("p (g e) -> p g e", e=H)
            nc.vector.tensor_copy(hbn_g[:, :, 1:W1], trh_v)
            nc.sync.dma_start(gather_in[:], hbn_n[:])
            nc.gpsimd.collective_compute(
                "AllGather", ALU.bypass, replica_groups=[list(range(NC))],
                ins=[gather_in.opt()], outs=[gather_out.opt()])
            gout_r = gather_out.rearrange("(c p) (b x) -> p b c x",
                                          p=128, b=B)
            nc.sync.dma_start(
                ghat.rearrange("p (b c x) -> p b c x", b=B, c=NC), gout_r)

            # ---------------- block 2 -------------------------------------
            ssrc2 = ssrc_calc(2, ghat_v, wrep2_v)
            out2T = wp.tile([H, B * GL], F32, name="out2T", tag="out2T")
            accs2 = [contraction(b, w2T_sb, sdb2, ssrc2, ghat_v)
                     for b in range(B)]
            for b in range(B):
                sl2 = slice(b * GL, (b + 1) * GL)
                post_acc(b, accs2[b], hbnT[:, sl2].bitcast(F32),
                         (hbnT[:, sl2], Wm2b),
                         Wn2a, Wn2b, Wm2a, Wm2b,
                         out2T[:, sl2], H)

            # transpose to natural and store
            tro = pp.tile([128, B * LCH * NO], F32, name="ps_tro", tag="trf",
                          bufs=1)
            for j in range(B * LCH):
                nc.tensor.transpose(tro[:, j * 32:(j + 1) * 32],
                                    out2T[:, j * 128:(j + 1) * 128],
                                    id_f[:])
            out_n = wp.tile([128, B * LCH * NO], F32, name="out_n",
                            tag="out_n")
            nc.vector.tensor_copy(out_n[:], tro[:])
            nc.sync.dma_start(
                out_r, out_n.rearrange("p (b l f) -> p b l f", b=B, l=LCH))

    nc.compile()
    return nc


def _prep_inputs(x, edges1, edges2, W_infer, b_infer, W_e1, b_e1, W_e2, b_e2,
                 W_n1, b_n1, W_n2, b_n2, W_m1, b_m1, W_m2, b_m2,
                 bn_gamma, bn_beta):
    f32 = np.float32
    bf16 = ml_dtypes.bfloat16
    x = np.asarray(x, f32)
    w1 = (ALPHA + (1.0 - ALPHA) * np.asarray(edges1, f32)).astype(bf16)
    w2 = (BETA + (1.0 - BETA) * np.asarray(edges2, f32)).astype(bf16)

    def wrepe(W_e, b_e):
        row = np.concatenate([[np.asarray(b_e, f32)[0]],
                              np.asarray(W_e, f32)[:H, 0]])
        return np.tile(row, (128, NCH)).astype(bf16)

    z = np.zeros((1, NO), f32)

    def stk(Wpart, brow):
        return np.concatenate([np.asarray(Wpart, f32), brow], 0)

    pack = np.zeros((128, PK_W), f32)
    pack[:, PK_WREP1:PK_WREP1 + NCH * W1] = wrepe(W_e1, b_e1)
    pack[:, PK_WREP2:PK_WREP2 + NCH * W1] = wrepe(W_e2, b_e2)
    pack[0:NI + 1, PK_WAUG:PK_WAUG + 32] = np.concatenate(
        [np.asarray(W_infer, f32), np.asarray(b_infer, f32)[None, :]], 0)
    pack[0:H, PK_WE1D] = np.asarray(W_e1, f32)[H:, 0]
    pack[0:H, PK_WE2D] = np.asarray(W_e2, f32)[H:, 0]
    blks = [
        (np.concatenate([z, np.asarray(W_n1, f32)[:H]], 0), 33),
        (stk(np.asarray(W_n1, f32)[H:], np.asarray(b_n1, f32)[None, :]), 33),
        (stk(np.asarray(W_m1, f32)[:H], np.asarray(b_m1, f32)[None, :]), 33),
        (np.asarray(W_m1, f32)[H:], 32),
        (np.concatenate([z, np.asarray(W_n2, f32)[:H]], 0), 33),
        (stk(np.asarray(W_n2, f32)[H:], np.asarray(b_n2, f32)[None, :]), 33),
        (stk(np.asarray(W_m2, f32)[:H], np.asarray(b_m2, f32)[None, :]), 33),
        (np.asarray(W_m2, f32)[H:], 32),
    ]
    packf = np.zeros((33, PF_W), f32)
    for i, (w, p) in enumerate(blks):
        pack[0:p, PK_W8 + i * 32:PK_W8 + (i + 1) * 32] = w
        packf[0:p, PF_W8 + i * 32:PF_W8 + (i + 1) * 32] = w
    packf[0, PF_ONER:PF_ONER + GL] = 1.0
    packf[:, PF_ONEC] = 1.0
    pack = pack.astype(bf16)

    in_maps = []
    for c in range(NC):
        sl = slice(c * GL, (c + 1) * GL)
        # per-core gene permutation for block 1: local genes first
        perm = np.concatenate([np.arange(c * GL, (c + 1) * GL),
                               np.arange(0, c * GL),
                               np.arange((c + 1) * GL, G)])
        xp = x[:, perm, :]                       # [B, G, NI] permuted
        xT = xp.transpose(2, 0, 1).reshape(NI, B * G)
        xT_b = np.concatenate([xT, np.ones((1, B * G), f32)], 0).astype(bf16)
        pf = packf.copy()
        pf[0, PF_BNG:PF_BNG + GL] = np.asarray(bn_gamma, f32)[sl]
        pf[0, PF_BNB:PF_BNB + GL] = np.asarray(bn_beta, f32)[sl]
        m = dict(
            xT_b=xT_b,
            w1T=np.ascontiguousarray(
                np.asarray(w1)[sl, :][:, perm].T),
            w2T=np.ascontiguousarray(np.asarray(w2)[sl, :].T),
            pack_b=pack,
            pack_f=pf,
        )
        in_maps.append(m)
    return in_maps


def kernel(**inputs):
    if "nc" not in _CACHE:
        _CACHE["nc"] = build_program()
    nc = _CACHE["nc"]
    in_maps = _prep_inputs(**inputs)
    res = run_bass_kernel_spmd(nc, in_maps, list(range(NC)))
    parts = [res.results[c]["out"].reshape(B, GL, NO) for c in range(NC)]
    return np.concatenate(parts, axis=1).astype(np.float32)
